# revision 37
# baseline (speedup 1.0000x reference)
"""Trainium2 Bass kernel for MultiHeadEdgeAttention (v4).

Sharding: 8 cores = 4 batches x 2 query-halves (256 queries each), no
collectives; each core produces a disjoint [256, 768] slice of the output.

Bottleneck history:
  v1 (243us): DMA-bound — gather-style edge DMA ran at 161GB/s (45% eff).
  v2 (133us): host pre-lays all layouts (DMA 334GB/s), but the kernel is
      LDWEIGHTS-bound: ~100K stationary columns @ ~1.2GHz = 83us on the PE
      weight port, 2/3 of it the edge stream (E slices as stationary).
  v4: cuts weight-port traffic: edge matmuls use fp8 DoubleRow (2 weights/
      cell -> K=256 per load, halving edge LDW columns + halving edge DMA),
      output matmuls flip to S-stationary (3K cols instead of 9K), softmax
      colsums ride the value matmul (ones-column fold), output bias rides a
      K=1 ones matmul, reciprocal uses the fast-approx DVE op.

Numerics: fp8(e4m3) is used ONLY for the edge stream (E and a quantized copy
of the attention weights); ~5% rms on the edge context, which is ~16% of the
output amplitude -> ~0.8% added output error. Value/score paths stay bf16.
All exact linear-algebra folds kept: Wke/Weo/Wo concat folds, bias folds via
sum(attn)==1 and softmax shift invariance, host-computed softcapped edge bias.
"""

import os
import numpy as np
import ml_dtypes

import concourse.bass as bass
from concourse import bacc
import concourse.mybir as mybir
from concourse.tile import TileContext
from contextlib import ExitStack

B, L, D, H, DE, DK = 4, 512, 768, 12, 64, 64
CAP = 5.0
NQ = 256                      # query rows per core
MC = 4                        # key chunks of 128
MP = 2                        # key chunk PAIRS (DoubleRow: K=256)
NCH = 8                       # edge chunks
CQ = NQ // NCH                # 32 queries per chunk
CP = CQ // 2                  # 16 query pairs per chunk
SM = (2.0 * DK) ** -0.5       # score scale
EBS = 2.0 ** -0.5             # edge bias scale
NCORE = 8
EDT = os.environ.get("EDT", "bf16")  # edge dtype: bf16 | fp8 (DoubleRow)

F32 = mybir.dt.float32
F32R = mybir.dt.float32r
BF16 = mybir.dt.bfloat16
FP8 = mybir.dt.float8e4
AF = mybir.ActivationFunctionType
ALU = mybir.AluOpType
PM = mybir.MatmulPerfMode
BF = ml_dtypes.bfloat16
F8 = ml_dtypes.float8_e4m3fn


def build():
    STG = int(os.environ.get("STG", "5"))
    nc = bacc.Bacc()

    qtin_d = nc.dram_tensor("qtin", (128, 6, NQ), BF16, kind="ExternalInput")
    ktin_d = nc.dram_tensor("ktin", (128, 6, L), BF16, kind="ExternalInput")
    vtin_d = nc.dram_tensor("vtin", (128, 6, L), BF16, kind="ExternalInput")
    wq_d = nc.dram_tensor("wq", (128, 6, D), BF16, kind="ExternalInput")
    wk_d = nc.dram_tensor("wk", (128, 6, D), BF16, kind="ExternalInput")
    wv_d = nc.dram_tensor("wv", (128, 6, D), BF16, kind="ExternalInput")
    ebt_d = nc.dram_tensor("ebt", (128, MC, 2, NQ), BF16, kind="ExternalInput")
    idt_d = nc.dram_tensor("ident", (128, 128), BF16, kind="ExternalInput")
    if EDT == "fp8":
        e_d = nc.dram_tensor("edge", (128, NCH, CP, MP, 2, 2, DE), FP8,
                             kind="ExternalInput")
    else:
        e_d = nc.dram_tensor("edge", (128, NCH, CP, MC, 2, DE), BF16,
                             kind="ExternalInput")
    wo_d = nc.dram_tensor("wo", (128, 12, D), BF16, kind="ExternalInput")
    bqs_d = nc.dram_tensor("bqs", (128, 6), F32, kind="ExternalInput")
    boutr_d = nc.dram_tensor("boutr", (1, D), BF16, kind="ExternalInput")
    out_d = nc.dram_tensor("outN", (2, 128, D), F32, kind="ExternalOutput")

    with TileContext(nc) as tc, ExitStack() as ctx:
        dpool = ctx.enter_context(tc.tile_pool(name="d", bufs=1))
        epool = ctx.enter_context(tc.tile_pool(name="e", bufs=2))
        opool = ctx.enter_context(tc.tile_pool(name="o", bufs=2))
        ppool = ctx.enter_context(tc.tile_pool(name="pp", bufs=2, space="PSUM"))

        # ---- weights + inputs, DMA'd in consumption order; wq split so
        # the first projection matmuls start as early as possible ----
        qtin = dpool.tile([128, 6, NQ], BF16)
        nc.sync.dma_start(out=qtin, in_=qtin_d[:, :, :])
        wq_sb = dpool.tile([128, 6, D], BF16)
        nc.sync.dma_start(out=wq_sb[:, :, 0:256], in_=wq_d[:, :, 0:256])
        nc.sync.dma_start(out=wq_sb[:, :, 256:768], in_=wq_d[:, :, 256:768])
        wk_sb = dpool.tile([128, 6, D], BF16)
        nc.sync.dma_start(out=wk_sb, in_=wk_d[:, :, :])
        ktin = dpool.tile([128, 6, L], BF16)
        nc.sync.dma_start(out=ktin, in_=ktin_d[:, :, :])
        wv_sb = dpool.tile([128, 6, D], BF16)
        nc.sync.dma_start(out=wv_sb, in_=wv_d[:, :, :])
        vtin = dpool.tile([128, 6, L], BF16)
        nc.sync.dma_start(out=vtin, in_=vtin_d[:, :, :])
        ebt_sb = dpool.tile([128, MC, 2, NQ], BF16)
        nc.sync.dma_start(out=ebt_sb, in_=ebt_d[:, :, :, :])
        idt_sb = dpool.tile([128, 128], BF16)
        nc.sync.dma_start(out=idt_sb, in_=idt_d[:, :])
        bqs = dpool.tile([128, 6], F32)
        nc.sync.dma_start(out=bqs, in_=bqs_d[:, :])
        boutr = dpool.tile([1, D], BF16)
        nc.sync.dma_start(out=boutr, in_=boutr_d[:, :])
        ech_tiles = []
        wo_sb = dpool.tile([128, 12, D], BF16)
        for c in range(NCH):
            if EDT == "fp8":
                ech = epool.tile([128, CP, MP, 2, 2, DE], FP8, tag="e")
            else:
                ech = epool.tile([128, CP, MC, 2, DE], BF16, tag="e")
            nc.sync.dma_start(out=ech, in_=e_d[:, c])
            ech_tiles.append(ech)
            if c == 5:  # wo needed only by the output matmuls at the tail
                nc.sync.dma_start(out=wo_sb, in_=wo_d[:, :, :])

        # ---- persistent activations ----
        kt_sb = dpool.tile([128, 6, L], BF16)     # k^T, head pairs stacked
        # qtz[:, 0] = q^T with odd-head rows zeroed, qtz[:, 1] = even zeroed;
        # adjacent so one N=512 matmul scores both heads of a pair
        qtz = dpool.tile([128, 2, 6, NQ], BF16)
        nc.vector.memset(qtz[64:128, 0, :, :], 0.0)
        nc.vector.memset(qtz[0:64, 1, :, :], 0.0)
        v_sb = dpool.tile([128, MC, H, DE + 1], BF16)  # col 64 = ones (colsum)
        nc.vector.memset(v_sb[:, :, :, DE:DE + 1], 1.0)
        pT = dpool.tile([128, MC, H, NQ], BF16)   # unnormalized exp scores S^T
        ctx_raw = dpool.tile([128, 6, NQ], F32)   # value ctx, pairs stacked
        ctx_sb = dpool.tile([128, 6, NQ], BF16)   # normalized ctx
        ecT = dpool.tile([128, 6, NQ], BF16)      # normalized edge ctx
        rbc = dpool.tile([128, 6, NQ], F32)       # 1/colsum, parity-aware
        cs_t = dpool.tile([1, H, NQ], F32R)       # colsums (f32r for PE bcast)
        mask_f = dpool.tile([1, 2, 128], F32)     # [lo-mask, hi-mask]
        nc.vector.memset(mask_f[:, 0, 0:64], 1.0)
        nc.vector.memset(mask_f[:, 0, 64:128], 0.0)
        nc.vector.memset(mask_f[:, 1, 0:64], 0.0)
        nc.vector.memset(mask_f[:, 1, 64:128], 1.0)
        mask_r = dpool.tile([1, 2, 128], F32R)
        nc.vector.tensor_copy(mask_r, mask_f)
        mlo, mhi = mask_r[:, 0, :], mask_r[:, 1, :]
        ones1 = dpool.tile([1, 128], BF16)
        nc.vector.memset(ones1, 1.0)
        if EDT == "fp8":
            # edge-only fp8 copy of pT, pre-laid as per-matmul stream blocks
            # [p, mp, nn, ko, q, par, hp] so each edge matmul's rhs slice is
            # contiguous; quantized on ScalarE right behind each exp
            pT8 = dpool.tile([128, MP, NQ // 2, 2, 2, 2, 6], FP8)

        # ---- phase 1+2 interleaved: projections + scores + exp ----
        # q/k projections for a head-half are emitted, then that half's score
        # matmuls + exps run while the other half's projections continue, so
        # the scalar-engine exp pipeline starts as early as possible.
        def proj_qk(t):
            ps_q = ppool.tile([128, NQ], F32, tag="sm", name=f"ps_q{t}")
            for kc in range(6):
                nc.tensor.matmul(ps_q, wq_sb[:, kc, t * 128:(t + 1) * 128],
                                 qtin[:, kc, :], start=(kc == 0), stop=(kc == 5))
            nc.vector.tensor_scalar(out=qtz[0:64, 0, t, :], in0=ps_q[0:64, :],
                                    scalar1=bqs[0:64, t:t + 1], scalar2=SM,
                                    op0=ALU.add, op1=ALU.mult)
            nc.vector.tensor_scalar(out=qtz[64:128, 1, t, :],
                                    in0=ps_q[64:128, :],
                                    scalar1=bqs[64:128, t:t + 1], scalar2=SM,
                                    op0=ALU.add, op1=ALU.mult)
            ps_k = ppool.tile([128, L], F32, tag="sm", name=f"ps_k{t}")
            for kc in range(6):
                nc.tensor.matmul(ps_k, wk_sb[:, kc, t * 128:(t + 1) * 128],
                                 ktin[:, kc, :], start=(kc == 0), stop=(kc == 5))
            nc.vector.tensor_copy(kt_sb[:, t, :], ps_k)

        def scores(hh):
            # edge bias preloaded into PSUM by identity matmuls; score
            # matmuls accumulate on top (one N=512 matmul per head pair)
            for mc in range(MC):
                ps_s = ppool.tile([128, 6, NQ], F32, tag="ss",
                                  name=f"ps_s{hh}{mc}")
                for k in range(3):
                    nc.tensor.matmul(
                        ps_s[:, 2 * k:2 * k + 2, :], idt_sb,
                        ebt_sb[:, mc], start=True, stop=False)
                for k in range(3):
                    hp = hh * 3 + k
                    nc.tensor.matmul(
                        ps_s[:, 2 * k:2 * k + 2, :],
                        kt_sb[:, hp, mc * 128:(mc + 1) * 128],
                        qtz[:, :, hp, :],
                        start=False, stop=True)
                nc.scalar.activation(pT[:, mc, hh * 6:(hh + 1) * 6, :], ps_s,
                                     AF.Exp)
                if EDT == "fp8":
                    nc.scalar.copy(
                        pT8[:, mc // 2, :, mc % 2, :, :, 3 * hh:3 * hh + 3],
                        pT[:, mc]
                        .rearrange("p (hp par) (nn q) -> p nn q par hp",
                                   par=2, q=2)[:, :, :, :, 3 * hh:3 * hh + 3])

        for t in (0, 1, 2):
            proj_qk(t)
        if STG >= 2:
            scores(0)
        for t in (3, 4, 5):
            proj_qk(t)
        for mc in range(MC):
            # shared stationary (vtin chunk) streams both 384-wide halves
            ps_v0 = ppool.tile([128, 384], F32, tag="sm", name=f"ps_v0{mc}")
            ps_v1 = ppool.tile([128, 384], F32, tag="sm", name=f"ps_v1{mc}")
            for kc in range(6):
                nc.tensor.matmul(ps_v0, vtin[:, kc, mc * 128:(mc + 1) * 128],
                                 wv_sb[:, kc, 0:384],
                                 start=(kc == 0), stop=(kc == 5))
                nc.tensor.matmul(ps_v1, vtin[:, kc, mc * 128:(mc + 1) * 128],
                                 wv_sb[:, kc, 384:768],
                                 start=(kc == 0), stop=(kc == 5))
            nc.vector.tensor_copy(v_sb[:, mc, 0:6, 0:DE],
                                  ps_v0.rearrange("p (h d) -> p h d", h=6))
            nc.vector.tensor_copy(v_sb[:, mc, 6:12, 0:DE],
                                  ps_v1.rearrange("p (h d) -> p h d", h=6))
        if STG >= 2:
            scores(1)

        # ---- phase 3: value stream + colsums (ones-column fold) ----
        for i in range(6 if STG >= 3 else 0):
            pv = ppool.tile([65, 2, NQ], F32, tag="sm")
            for k in range(2):
                h = 2 * i + k
                for mc in range(MC):
                    nc.tensor.matmul(pv[:, k, :], v_sb[:, mc, h, :],
                                     pT[:, mc, h, :],
                                     start=(mc == 0), stop=(mc == MC - 1))
            nc.vector.tensor_copy(cs_t[0:1, 2 * i:2 * i + 2, :], pv[64:65, :, :])
            for k in range(2):
                h = 2 * i + k
                par, hp = h % 2, h // 2
                nc.scalar.copy(ctx_raw[64 * par:64 * par + 64, hp, :],
                               pv[0:64, k, :])

        # ---- normalizer: rbc[p, hp, n] = 1/cs[2*hp + (p>=64), n] ----
        cs_v = cs_t.rearrange("p (hp two) n -> p hp two n", two=2)
        for c in range(3 if STG >= 3 else 0):
            pb = ppool.tile([128, 2, NQ], F32, tag="sm")
            nc.tensor.matmul(pb, mlo, cs_v[0:1, 2 * c:2 * c + 2, 0, :],
                             start=True, stop=False)
            nc.tensor.matmul(pb, mhi, cs_v[0:1, 2 * c:2 * c + 2, 1, :],
                             start=False, stop=True)
            nc.vector.reciprocal_approx_fast(rbc[:, 2 * c:2 * c + 2, :], pb)
        if STG >= 3:
            nc.vector.tensor_mul(ctx_sb, ctx_raw, rbc)
        # parity-swapped copy: rbc_x[p] = rbc[p^64]; lets the edge extraction
        # pick an rbc operand whose base partition matches its psum-copy input
        rbc_x = dpool.tile([128, 6, NQ], F32)
        if STG >= 3:
            nc.vector.tensor_copy(rbc_x[0:64], rbc[64:128])
            nc.vector.tensor_copy(rbc_x[64:128], rbc[0:64])

        # ---- phase 4: edge stream ----
        rbc_v = rbc.rearrange("p hp (nn two) -> p hp nn two", two=2)
        rbcx_v = rbc_x.rearrange("p hp (nn two) -> p hp nn two", two=2)
        ecT_v = ecT.rearrange("p hp (nn two) -> p hp nn two", two=2)
        for c in range(NCH if STG >= 4 else 0):
            ech = ech_tiles[c]
            n0 = c * CQ
            pe_t = ppool.tile([128, CP, 2, 2, 6], F32, tag="sm")
            for pl in range(CP):
                if EDT == "fp8":
                    for mp in range(MP):
                        nc.tensor.matmul(
                            pe_t[:, pl].rearrange("p a b c -> p (a b c)"),
                            ech[:, pl, mp].rearrange("p a b c -> p a (b c)"),
                            pT8[:, mp, (n0 // 2) + pl]
                            .rearrange("p a b c d -> p a (b c d)"),
                            start=(mp == 0), stop=(mp == MP - 1),
                            perf_mode=PM.DoubleRow)
                else:
                    for mc in range(MC):
                        nc.tensor.matmul(
                            pe_t[:, pl].rearrange("p a b c -> p (a b c)"),
                            ech[:, pl, mc].rearrange("p a b -> p (a b)"),
                            pT[:, mc, :, n0 + 2 * pl:n0 + 2 * pl + 2]
                            .rearrange("p (hp two) n -> p n two hp", two=2),
                            start=(mc == 0), stop=(mc == MC - 1))
            ec_raw = epool.tile([128, CP, 2, 2, 6], F32, tag="er")
            nc.scalar.copy(ec_raw, pe_t)
            for q in range(2):
                for par in range(2):
                    rsel = rbc_v if par == q else rbcx_v
                    nc.vector.tensor_mul(
                        ecT_v[64 * par:64 * par + 64, :,
                              c * CP:(c + 1) * CP, q]
                        .rearrange("p a b -> p b a"),
                        ec_raw[64 * q:64 * q + 64, :, q, par, :],
                        rsel[64 * q:64 * q + 64, :,
                             c * CP:(c + 1) * CP, q]
                        .rearrange("p a b -> p b a"))

        # ---- phase 5: output matmuls, S-stationary ----
        # out[n, o] = sum_j S_j[(hd), n]^T @ wo_j[(hd), o], + bias via K=1 MM
        for nq in range(2 if STG >= 5 else 0):
            for half in range(2):
                po = ppool.tile([128, 384], F32, tag="sm")
                nc.tensor.matmul(po, ones1,
                                 boutr[:, half * 384:(half + 1) * 384],
                                 start=True, stop=False)
                for j in range(6):
                    nc.tensor.matmul(
                        po, ctx_sb[:, j, nq * 128:(nq + 1) * 128],
                        wo_sb[:, j, half * 384:(half + 1) * 384],
                        start=False, stop=False)
                for j in range(6):
                    nc.tensor.matmul(
                        po, ecT[:, j, nq * 128:(nq + 1) * 128],
                        wo_sb[:, 6 + j, half * 384:(half + 1) * 384],
                        start=False, stop=(j == 5))
                ot = opool.tile([128, 384], F32, tag="ot")
                nc.scalar.copy(ot, po)
                nc.sync.dma_start(
                    out=out_d[nq, :, half * 384:(half + 1) * 384], in_=ot)
        if STG < 5:  # still produce an output tensor so the NEFF has one
            zt = opool.tile([128, 384], F32, tag="ot")
            nc.vector.memset(zt, 0.0)
            for nq in range(2):
                for half in range(2):
                    nc.sync.dma_start(
                        out=out_d[nq, :, half * 384:(half + 1) * 384], in_=zt)
    nc.compile()
    return nc


def host_prep(inputs):
    """Build the 8 per-core input maps from full inputs (all layouts pre-laid
    so every DMA moves large contiguous per-partition lines)."""
    Q, K, V = inputs["Q"], inputs["K"], inputs["V"]
    E = inputs["edge_embs"]
    Wq, bq = inputs["Wq"], inputs["bq"]
    Wk = inputs["Wk"]
    Wv, bv = inputs["Wv"], inputs["bv"]
    Wke, bke = inputs["Wke"], inputs["bke"]
    We, be = inputs["We"], inputs["be"]
    Weo, beo = inputs["Weo"], inputs["beo"]
    Wo, bo = inputs["Wo"], inputs["bo"]

    Wo1, Wo2 = Wo[:D], Wo[D:]
    M = (Weo @ Wo2).astype(np.float32)                       # [768, 768]
    Mh = M.reshape(H, DE, D)
    wec = np.concatenate([Wke @ Mh[h] for h in range(H)], axis=0)
    bout_full = (bo + bv @ Wo1 + bke @ Mh.sum(0) + beo @ Wo2).astype(np.float32)

    bqs = np.ascontiguousarray(np.asarray(bq, np.float32)
                               .reshape(6, 128).T).astype(np.float32)
    boutr = np.ascontiguousarray(bout_full[None, :]).astype(BF)

    def lay_w(W):        # [768 in, 768 out] -> [128, 6 kc, 768]
        return np.ascontiguousarray(
            np.asarray(W, np.float32).reshape(6, 128, D)
            .transpose(1, 0, 2)).astype(BF)
    wq_b, wk_b, wv_b = lay_w(Wq), lay_w(Wk), lay_w(Wv)

    def lay_wo(Wx):      # [768 (h*64+d), 768] -> [128 (par*64+d), 6 hp, 768]
        t = np.asarray(Wx, np.float32).reshape(6, 2, DE, D)
        return t.transpose(1, 2, 0, 3).reshape(128, 6, D)
    wo_b = np.ascontiguousarray(
        np.concatenate([lay_wo(Wo1), lay_wo(wec)], axis=1)).astype(BF)

    We1 = np.asarray(We, np.float32)[:, 0]
    ident = np.eye(128, dtype=np.float32).astype(BF)
    in_maps = []
    for core in range(NCORE):
        b, half = core // 2, core % 2
        n0 = half * NQ
        Qs = np.asarray(Q[b, n0:n0 + NQ], np.float32)        # [256, 768]
        Es = np.asarray(E[b, n0:n0 + NQ], np.float32)        # [256, 512, 64]
        raw = (Es @ We1 + float(be[0])) * EBS                # [256, 512]
        ebt = (CAP * np.tanh(raw / CAP)).T                   # [512, 256]
        ebt_l = np.ascontiguousarray(np.repeat(
            ebt.reshape(MC, 128, NQ).transpose(1, 0, 2)[:, :, None, :],
            2, axis=2)).astype(BF)

        def lay_in(X, n):  # [n, 768] -> [128, 6 kc, n]  (X^T chunked)
            return np.ascontiguousarray(
                X.T.reshape(6, 128, n).transpose(1, 0, 2)).astype(BF)

        if EDT == "fp8":
            # [p, c, pl, mp, ko, q, d]; n = 32c + 2pl + q, m = (2mp+ko)*128 + p
            e8 = np.ascontiguousarray(
                Es.reshape(NCH, CP, 2, MP, 2, 128, DE)
                .transpose(5, 0, 1, 3, 4, 2, 6)).astype(F8)
        else:
            e8 = np.ascontiguousarray(
                Es.reshape(NCH, CP, 2, MC, 128, DE)
                .transpose(4, 0, 1, 3, 2, 5)).astype(BF)
        in_maps.append({
            "qtin": lay_in(Qs, NQ),
            "ktin": lay_in(np.asarray(K[b], np.float32), L),
            "vtin": lay_in(np.asarray(V[b], np.float32), L),
            "wq": wq_b, "wk": wk_b, "wv": wv_b,
            "ebt": ebt_l, "ident": ident, "edge": e8, "wo": wo_b,
            "bqs": bqs, "boutr": boutr,
        })
    return in_maps


def kernel(**inputs):
    from concourse.bass_utils import run_bass_kernel_spmd
    in_maps = host_prep(inputs)
    nc = build()
    res = run_bass_kernel_spmd(nc, in_maps, core_ids=list(range(NCORE)))
    out = np.empty((B, L, D), np.float32)
    for core in range(NCORE):
        b, half = core // 2, core % 2
        out[b, half * NQ:(half + 1) * NQ] = res.results[core]["outN"].reshape(NQ, D)
    return out
